# revision 19
# baseline (speedup 1.0000x reference)
"""Neural ODE (explicit Euler, 20 steps) Trainium2 Bass kernel — fp8 DoubleRow.

z_{s+1} = z_s + h * (tanh(z_s @ W1 + b1) @ W2 + b2),  z0: [8192, 512] f32.

Pure data parallel over 8 NeuronCores (1024 batch rows each); state kept
feature-major (zT: [512 features, 1024 batch]) resident in SBUF for all 20
steps. Matmuls run in fp8 e4m3 with perf_mode=DoubleRow (K=256 per matmul,
2 fp8 weights per PE cell), halving the matmul count vs fp16: per step per
core 32 DoubleRow MMs (mm1 + mm2, each 2 chunks x 4 output tiles x 2 K-pairs).

Accuracy plan (measured rel_max ~7.7e-3 vs 2e-2 budget):
 - fp16 master state zm, updated from fp32 PSUM each step (fp16 ulp ~4e-3
   random-walks to ~1e-2 abs over 20 steps — negligible vs the fp8 terms);
   fresh fp8 quantization of z per step; fp16 output DMA (host upcasts).
 - weights quantized with *error feedback* over NCOPIES=5 cycling copies:
   copy_i = q8(W - sum_of_previous_copy_errors), so partial sums of the
   systematic weight-quantization error stay bounded (~1 quantization step)
   instead of growing linearly over 20 steps.
 - power-of-2 scaling keeps everything in e4m3 normal range: W1 grid = W1*2^9
   (descaled by the tanh's scale=2^-9), W2 grid = W2*(h*2^12) (descaled by
   the state update's fused scalar 2^-12, which also applies h).

Schedule (see SCHED; forms searched on hardware):
 - per-step engine loads: DVE 6 STT ~1213ns each, ACT 4 tanh ~1113 +
   2 zr-copies ~1145; PE 32 MMs at warm cadence ~215ns. The ACT queue tail
   (tanh x4 then the two c0 zr re-quantizes, which gate the next step's
   first mm1) sets the ~8.4us steady-state period; every attempt to shorten
   it overloads DVE (engine-capacity deadlock, see notes below).
 - c1h1 uses dirI (zm STT immediately after zr rather than deferred):
   measured -1.5us vs deferring both c1 zms.
 - PSUM budget is the hard wall: 8 banks = ph 2-buf [128,2,512] (4) +
   py 2-buf (4). Persistent-PSUM state accumulation (mm2 start=False onto
   a live bank, which would delete zm STTs) needs 2 banks per granule and
   cannot fit alongside double-buffered ph+py; every 1-bank retiling
   variant (split tanh, py subtiles) is tanh-rate- or WAR-stall-bound.
   Matmul bf16 PSUM output (1-bank granules) is TRN3-only.

The steady-state period is CONSERVED at ~8.61us across every reachable
schedule: the ring mm1_00 -(PE ~0.9us)-> tanh00 -> tanh01 -> [parallel,
balanced: ACT {tanh10,tanh11,zr00} ~3.3us || PE {mm2_00,mm2_01} 1.7us +
DVE zm01 1.3us] -> zr01 -> next mm1_00. K=512 contraction spans force
whole-granule deps, so both branches must shrink together, and DVE/ACT
have no spare capacity to absorb moved ops. Measured dead ends:
all-mm1-first PE order +7us; both c0 zr copies on DVE CAST (650ns, 2x
mode — works on DVE, NOT on ACT) overloads DVE +6us; g00/g01 sC +8-16us;
half-granule mm1 fillers: no period change; splitting a zr tile across
ACT+DVE writer engines loses 3-6us; GPSIMD f32->f8 CAST 3.6us/FD1024.
Device note: sustained back-to-back runs trip a chip-wide P0 downclock
(2.4->2.0 GHz, all engines ~x1.2 slower); cool-gap measurements required.

Hot path assumes b1 = b2 = 0 (true for this problem's inputs); a general
with_bias build is compiled lazily only if nonzero biases ever show up.
"""

import numpy as np
import ml_dtypes

P = 128
D = 512
B_FULL = 8192
NCORES = 8
BSH = B_FULL // NCORES  # 1024 batch rows per core
NSTEPS = 20
FT = D // P             # 4 feature tiles
CB = 512                # batch columns per chunk
NCHUNK = BSH // CB      # 2 chunks
NWARM = 4               # data-independent PE prewarm matmuls (HAM clock ramp).
                        # With the z16 seed DMA split/reordered by need, the
                        # earlier step-0 start no longer stalls on z16 (the
                        # failure mode that made NWARM<8 regress before)
NCOPIES = 5             # error-feedback fp8 weight copies (cycled over steps)
SW1 = 512.0             # W1 fp8 grid scale (descaled via tanh scale)
SW2 = 4096.0            # W2 fp8 grid carries h*SW2; descaled by C2 in DVE
C1 = 1.0 / SW1
C2 = 1.0 / SW2

# ---- schedule configuration (overridable via SCHED_JSON env for search) ----
import os as _os
import json as _json

SCHED = {
    # PE group emission order: ("m1"|"m2", chunk, half)
    "pe_order": [("m1", 0, 0), ("m1", 0, 1), ("m1", 1, 0), ("m2", 0, 0),
                 ("m1", 1, 1), ("m2", 0, 1), ("m2", 1, 0), ("m2", 1, 1)],
    # state-update form per granule (c,h): "sA"|"sC"|"dir"|"dirI"
    "forms": {(0, 0): "sA", (0, 1): "sA", (1, 0): "dir", (1, 1): "dirI"},
}
if _os.environ.get("SCHED_JSON"):
    _sj = _json.loads(_os.environ["SCHED_JSON"])
    if "pe_order" in _sj:
        SCHED["pe_order"] = [tuple(x) for x in _sj["pe_order"]]
    if "forms" in _sj:
        SCHED["forms"] = {(int(k[0]), int(k[2])): v
                          for k, v in _sj["forms"].items()}
    for _k in ("force", "act_chain", "dve_chain"):
        if _k in _sj:
            SCHED[_k] = _sj[_k]

_CACHE = {}


def _build_nc(with_bias):
    import concourse.bacc as bacc
    import concourse.mybir as mybir
    import concourse.tile as tile

    f32 = mybir.dt.float32
    f16 = mybir.dt.float16
    bf16 = mybir.dt.bfloat16
    f8 = mybir.dt.float8e4
    Tanh = mybir.ActivationFunctionType.Tanh
    Copy = mybir.ActivationFunctionType.Copy
    DR = mybir.MatmulPerfMode.DoubleRow
    MUL = mybir.AluOpType.mult
    ADD = mybir.AluOpType.add

    from concourse.tile import add_dep_helper

    nc = bacc.Bacc("TRN2", target_bir_lowering=False, debug=False)
    # z transposed on host: [D, BSH] feature-major
    z8_in = nc.dram_tensor("z8", [D, BSH], f8, kind="ExternalInput")
    z16_in = nc.dram_tensor("z16", [D, BSH], f16, kind="ExternalInput")
    w1_in = [
        nc.dram_tensor(f"w1_{i}", [P, FT, D], f8, kind="ExternalInput")
        for i in range(NCOPIES)
    ]
    w2_in = [
        nc.dram_tensor(f"w2_{i}", [P, FT, D], f8, kind="ExternalInput")
        for i in range(NCOPIES)
    ]
    if with_bias:
        # biases[p, jt, s] = b1[jt*128+p] + s * (W1^T (h*b2))[jt*128+p]
        b_in = nc.dram_tensor("biases", [P, FT, NSTEPS], f32, kind="ExternalInput")
        # bfin[p, jt] = NSTEPS * h * b2[jt*128+p] / C2
        bf_in = nc.dram_tensor("bfin", [P, FT], f32, kind="ExternalInput")
    z_out = nc.dram_tensor("zout", [D, BSH], f16, kind="ExternalOutput")

    z8_t = z8_in.ap().rearrange("(ft p) b -> p ft b", p=P)
    z16_t = z16_in.ap().rearrange("(ft p) b -> p ft b", p=P)
    zout_t = z_out.ap().rearrange("(ft p) b -> p ft b", p=P)

    def cslice(c):
        return slice(c * CB, (c + 1) * CB)

    with tile.TileContext(nc) as tc:
        with (
            tc.tile_pool(name="wpool", bufs=1) as wpool,
            tc.tile_pool(name="zpool", bufs=3) as zpool,
            tc.tile_pool(name="zrpool", bufs=3) as zrpool,
            tc.tile_pool(name="apool", bufs=3) as apool,
            tc.tile_pool(name="php", bufs=2, space="PSUM") as php,
            tc.tile_pool(name="pyp", bufs=2, space="PSUM") as pyp,
        ):
            # ---- PE prewarm with real fp8 DoubleRow matmuls (transpose-mode
            # does NOT count as PE-busy for the HAM clock monitor, so warm
            # with the same instruction type the steps use; ramps the clock
            # to 2.4 GHz while the input DMAs run) ----
            wtile = wpool.tile([P, 2, CB], f8, tag="wtile")
            nc.vector.memset(wtile[:], 1.0)
            warm_sink = wpool.tile([P, P], f32, tag="warm")
            # preload the tanh ACT table set while DMAs run
            nc.scalar.activation(
                warm_sink[0:1, 0:1], wtile[0:1, 0, 0:1], Tanh,
            )
            # (warming in the py pool instead measured ~1us worse on average
            # — it perturbs the py-ring phase entering step 0)
            for i in range(NWARM):
                wps = php.tile([P, 2, CB], f32, tag="ph", name=f"warm{i}")
                nc.tensor.matmul(
                    wps[:, 0, :], wtile[:, :, 0:P], wtile[:],
                    start=True, stop=True, perf_mode=DR,
                )
                if i == 0:
                    # consume the first warm tile only: a reader on the LAST
                    # warm MM's psum was measured to delay step 0 by ~0.9us
                    # (its copy gated the slot step 0's first matmul needs)
                    nc.vector.tensor_copy(warm_sink[:], wps[:, 0, 0:P])

            # ---- input DMAs, ordered by first use ----
            z8sb = wpool.tile([P, FT, BSH], f8, tag="z8")
            w1sb = [
                wpool.tile([P, FT, D], f8, tag=f"w1_{i}", name=f"w1_{i}")
                for i in range(NCOPIES)
            ]
            w2sb = [
                wpool.tile([P, FT, D], f8, tag=f"w2_{i}", name=f"w2_{i}")
                for i in range(NCOPIES)
            ]
            # first-use-granular startup DMAs: step 0's first MMs need only
            # z8 chunk0/kp0 + the w1_0 slice for kp0; splitting lets the PE
            # start ~2us earlier than one monolithic 256KB+256KB pair
            nc.sync.dma_start(z8sb[:, 0:2, cslice(0)], z8_t[:, 0:2, cslice(0)])
            nc.sync.dma_start(w1sb[0][:, 0:2, :], w1_in[0].ap()[:, 0:2, :])
            nc.sync.dma_start(z8sb[:, 2:4, cslice(0)], z8_t[:, 2:4, cslice(0)])
            nc.sync.dma_start(w1sb[0][:, 2:4, :], w1_in[0].ap()[:, 2:4, :])
            # z16 (fp16 master seed) split into quarter-slices and
            # interleaved by first-use time: step 0's DVE state ops need
            # z16_c0 halves ~2.6us after step 0 starts — a single 1MB z16
            # DMA queued later lands at ~14us and gates the whole chain
            # (this was why earlier step-0 starts regressed)
            z16sb = wpool.tile([P, FT, BSH], f16, tag="z16")
            nc.sync.dma_start(
                z16sb[:, 0:2, cslice(0)], z16_t[:, 0:2, cslice(0)])
            nc.sync.dma_start(z8sb[:, :, cslice(1)], z8_t[:, :, cslice(1)])
            nc.sync.dma_start(w2sb[0][:], w2_in[0].ap())
            nc.sync.dma_start(
                z16sb[:, 2:4, cslice(0)], z16_t[:, 2:4, cslice(0)])
            nc.sync.dma_start(
                z16sb[:, 0:2, cslice(1)], z16_t[:, 0:2, cslice(1)])
            nc.sync.dma_start(
                z16sb[:, 2:4, cslice(1)], z16_t[:, 2:4, cslice(1)])
            if with_bias:
                bias_sb = wpool.tile([P, FT, NSTEPS], f32, tag="bias")
                nc.sync.dma_start(bias_sb[:], b_in.ap())
                bfin_sb = wpool.tile([P, FT], f32, tag="bfin")
                nc.sync.dma_start(bfin_sb[:], bf_in.ap())
            for i in range(1, NCOPIES):
                nc.sync.dma_start(w1sb[i][:], w1_in[i].ap())
                nc.sync.dma_start(w2sb[i][:], w2_in[i].ap())

            # state kept as 2-bank-pair granules: [chunk][half] where half h
            # covers feature tiles {2h, 2h+1} (= K-pair h for matmul rhs)
            def hslice(h):
                return slice(2 * h, 2 * h + 2)

            zr_cur = [[z8sb[:, hslice(h), cslice(c)] for h in range(2)]
                      for c in range(NCHUNK)]
            zm_cur = [[z16sb[:, hslice(h), cslice(c)] for h in range(2)]
                      for c in range(NCHUNK)]

            # ---- 20 Euler steps (schedule is config-driven; see SCHED) ----
            # Forms per granule (c,h):
            #   "sA": serial, DVE zm then ACT re-quantize zr (from f16 zm)
            #   "sC": serial, DVE zm then DVE CAST zr (f16->f8, 2x mode
            #         ~650ns vs ACT copy 1145ns)
            #   "dir": DVE STT zr straight from PSUM; zm deferred to the
            #          zm_deferred tail (py-slot reuse gates next mm2!)
            #   "dirI": direct zr then immediate zm (not deferred)
            for s in range(NSTEPS):
                wi = s % NCOPIES
                last = s == NSTEPS - 1
                a_t = {}

                ph_t = {}

                def emit_mm1_half(c, h, jl):
                    # one jl sub-pair (2 MMs); lets a half-granule slot into
                    # the PE hole before mm2_00 (which waits on tanh01)
                    if jl == 0:
                        ph_t[(c, h)] = php.tile([P, 2, CB], f32, tag="ph",
                                                name=f"ph{s}_{c}_{h}")
                    ph = ph_t[(c, h)]
                    jt = 2 * h + jl
                    for kp in range(2):
                        nc.tensor.matmul(
                            ph[:, jl, :],
                            w1sb[wi][:, hslice(kp), jt * P:(jt + 1) * P],
                            zr_cur[c][kp],
                            start=(kp == 0), stop=(kp == 1),
                            perf_mode=DR,
                        )
                    if jl == 1:
                        a = apool.tile([P, 2, CB], f8, tag=f"a{c}_{h}",
                                       name=f"a{s}_{c}_{h}")
                        if with_bias:
                            for j2 in range(2):
                                hi = nc.scalar.activation(
                                    a[:, j2, :], ph[:, j2, :], Tanh,
                                    bias=bias_sb[:, 2 * h + j2, s:s + 1],
                                    scale=C1,
                                )
                        else:
                            hi = nc.scalar.activation(
                                a[:], ph[:], Tanh, scale=C1)
                        act_h[("tanh", c, h)] = hi
                        a_t[(c, h)] = a

                def emit_mm1(c, h):
                    emit_mm1_half(c, h, 0)
                    emit_mm1_half(c, h, 1)

                def emit_mm2(c, h):
                    py = pyp.tile([P, 2, CB], f32, tag="py",
                                  name=f"py{s}_{c}_{h}")
                    for jl in range(2):
                        jt2 = 2 * h + jl
                        for kp in range(2):
                            nc.tensor.matmul(
                                py[:, jl, :],
                                w2sb[wi][:, hslice(kp), jt2 * P:(jt2 + 1) * P],
                                a_t[(c, kp)][:],
                                start=(kp == 0), stop=(kp == 1),
                                perf_mode=DR,
                            )
                    zm_new = zpool.tile([P, 2, CB], f16,
                                        tag=f"z{c}_{h}",
                                        name=f"zm{s}_{c}_{h}")
                    if not last:
                        form = SCHED["forms"][(c, h)]
                        zr_new = zrpool.tile([P, 2, CB], f8, tag=f"zr{c}_{h}",
                                             name=f"zr{s}_{c}_{h}")
                        if form in ("sA", "sC"):
                            dve_h[("zm", c, h)] = nc.vector.scalar_tensor_tensor(
                                zm_new[:], py[:], C2, zm_cur[c][h],
                                MUL, ADD,
                            )
                            if form == "sA":
                                act_h[("zr", c, h)] = nc.scalar.activation(
                                    zr_new[:], zm_new[:], Copy, scale=1.0,
                                )
                            else:
                                dve_h[("zr", c, h)] = nc.vector.tensor_copy(
                                    zr_new[:], zm_new[:])
                        else:
                            dve_h[("zr", c, h)] = nc.vector.scalar_tensor_tensor(
                                zr_new[:], py[:], C2, zm_cur[c][h],
                                MUL, ADD,
                            )
                            if form == "dirI":
                                dve_h[("zm", c, h)] = nc.vector.scalar_tensor_tensor(
                                    zm_new[:], py[:], C2, zm_cur[c][h],
                                    MUL, ADD,
                                )
                            else:
                                zm_deferred.append(
                                    (py, zm_new, zm_cur[c][h], (c, h)))
                        zr_cur[c][h] = zr_new[:]
                        zm_cur[c][h] = zm_new[:]
                    else:
                        if with_bias:
                            tmp = zpool.tile([P, 2, CB], f32,
                                             tag=f"tmp{c}_{h}",
                                             name=f"tmp{s}_{c}_{h}")
                            for jl in range(2):
                                jt2 = 2 * h + jl
                                nc.vector.tensor_scalar(
                                    tmp[:, jl, :], py[:, jl, :],
                                    C2, bfin_sb[:, jt2:jt2 + 1], MUL, ADD,
                                )
                            nc.vector.tensor_add(
                                zm_new[:], tmp[:], zm_cur[c][h])
                        else:
                            nc.vector.scalar_tensor_tensor(
                                zm_new[:], py[:], C2, zm_cur[c][h],
                                MUL, ADD,
                            )
                        nc.sync.dma_start(
                            zout_t[:, hslice(h), cslice(c)], zm_new[:])

                zm_deferred = []
                act_h = {}
                dve_h = {}
                for kind, c, h in SCHED["pe_order"]:
                    if kind == "m1":
                        emit_mm1(c, h)
                    elif kind == "m1a":
                        emit_mm1_half(c, h, 0)
                    elif kind == "m1b":
                        emit_mm1_half(c, h, 1)
                    else:
                        emit_mm2(c, h)
                for py, zm_new, zm_old, gh in zm_deferred:
                    dve_h[("zm",) + gh] = nc.vector.scalar_tensor_tensor(
                        zm_new[:], py[:], C2, zm_old, MUL, ADD,
                    )
                if SCHED.get("force") and not last:
                    # force per-engine issue order so the list scheduler
                    # cannot scramble the intended ring schedule
                    for chain, hmap in ((SCHED["act_chain"], act_h),
                                        (SCHED["dve_chain"], dve_h)):
                        prev = None
                        for key in chain:
                            cur = hmap.get(tuple(key))
                            if cur is None:
                                continue
                            if prev is not None:
                                add_dep_helper(
                                    cur.ins, prev.ins,
                                    reason="forced engine order")
                            prev = cur

    nc.finalize()
    return nc


def _get_nc(with_bias):
    key = ("nc", with_bias)
    if key not in _CACHE:
        _CACHE[key] = _build_nc(with_bias)
    return _CACHE[key]


def _q8(x):
    """fp32/64 -> TRN e4m3 (max +-240) with RNE, as ml_dtypes.float8_e4m3."""
    return np.clip(np.asarray(x, dtype=np.float32), -240.0, 240.0).astype(
        ml_dtypes.float8_e4m3
    )


def _feedback_copies(W, scale, n):
    """n fp8 copies of W*scale with error feedback: partial sums of the
    per-copy quantization errors stay bounded by ~one quantization step."""
    Wd = W.astype(np.float64) * scale
    cum = np.zeros_like(Wd)
    out = []
    for _ in range(n):
        q = _q8(Wd - cum)
        out.append(q)
        cum = cum + (q.astype(np.float64) - Wd)
    return out


def _tile_w(q):
    # [D, D] (k, j) -> [P, FT, D] with k = kt*128 + p
    return np.ascontiguousarray(q.reshape(FT, P, D).transpose(1, 0, 2))


def _prepare_inputs(z0, t, W1, b1, W2, b2):
    z0 = np.asarray(z0, dtype=np.float32)
    t = np.asarray(t, dtype=np.float64)
    W1 = np.asarray(W1, dtype=np.float64)
    b1 = np.asarray(b1, dtype=np.float64)
    W2 = np.asarray(W2, dtype=np.float64)
    b2 = np.asarray(b2, dtype=np.float64)

    h = (float(t[1]) - float(t[0])) / NSTEPS
    with_bias = bool(np.any(b1 != 0.0) or np.any(b2 != 0.0))

    zT = np.ascontiguousarray(z0.T)                    # [D, B_FULL] f32
    zT16 = zT.astype(np.float16)
    zT8 = _q8(zT)

    w1c = [_tile_w(q) for q in _feedback_copies(W1, SW1, NCOPIES)]
    w2c = [_tile_w(q) for q in _feedback_copies(W2, h * SW2, NCOPIES)]

    shared = {}
    for i in range(NCOPIES):
        shared[f"w1_{i}"] = w1c[i]
        shared[f"w2_{i}"] = w2c[i]
    if with_bias:
        b2h = b2 * h
        wtb = W1.T @ b2h  # [D]
        biases = np.stack(
            [b1 + s * wtb for s in range(NSTEPS)], axis=0
        ).astype(np.float32)  # [NSTEPS, D]
        shared["biases"] = np.ascontiguousarray(
            biases.reshape(NSTEPS, FT, P).transpose(2, 1, 0)
        )
        shared["bfin"] = np.ascontiguousarray(
            (NSTEPS * b2h / C2).astype(np.float32).reshape(FT, P).T
        )

    in_maps = []
    for i in range(NCORES):
        m = {
            "z8": np.ascontiguousarray(zT8[:, i * BSH:(i + 1) * BSH]),
            "z16": np.ascontiguousarray(zT16[:, i * BSH:(i + 1) * BSH]),
        }
        m.update(shared)
        in_maps.append(m)
    return in_maps


def _run(in_maps, trace=False):
    from concourse import bass_utils

    nc = _get_nc("biases" in in_maps[0])
    res = bass_utils.run_bass_kernel_spmd(
        nc, in_maps, core_ids=list(range(NCORES)), trace=trace,
    )
    return res


def kernel(z0, t, W1, b1, W2, b2):
    in_maps = _prepare_inputs(z0, t, W1, b1, W2, b2)
    res = _run(in_maps)
    outT = np.concatenate([np.asarray(r["zout"], dtype=np.float32) for r in res.results], axis=1)  # [D, B]
    return np.ascontiguousarray(outT.T).astype(np.float32)

